# revision 8
# baseline (speedup 1.0000x reference)
"""Trainium2 Bass kernel for nn_DifferentiableRocket.

Model: y = [max_pool ‖ mean_pool](conv1d(x, kernels)) @ W.T + b
  x [64,1,2048] f32, kernels [2000,1,9], W [10,4000], b [10] -> out [64,10]

Sharding: kernel-axis tensor parallel — each of 8 cores owns 250 conv
filters and the matching classifier columns; partial logits are summed on
the host (cheaper than an on-device all-reduce for a [10,64] tile).

Per-core device algorithm:
  * conv as 4x row-tiled PE matmuls (contraction = 9 taps, one 32-row
    group per lo-block of 512), weights stationary, out in PSUM
    [128 nk, 2048 lo] per (batch, nk-block) unit.
  * max-pool: ONE DVE tensor_tensor_reduce per unit:
        fold = max(psum[:, :1024], psum[:, 1024:2048])
        maxfeat[:, b] = reduce_max(fold)
    which drains PSUM at the 2-elem/lane/cycle ceiling.
  * mean-pool is algebraically collapsed: mean-part of the logits equals
    (S @ M.T) where S[b,k] = sum_lo x[b, lo+k] (9 windowed sums, computed
    on the idle ACT engine via activation accum_out) and
    M[c,k] = sum_nk W[c,2nk+1] * kernels[nk,k] / 2040 (host-precomputed
    weight packing; bias/8 folded in as a 10th row).
  * logits.T [10,64] = W_max.T-matmuls over maxfeat + M_aug @ S_aug.T,
    accumulated in one PSUM tile, DMA'd out; host sums the 8 cores.
"""

import sys

sys.path.insert(0, "/opt/trn_rl_repo")

from contextlib import ExitStack

import numpy as np

import concourse.bacc as bacc
import concourse.bass as bass
import concourse.mybir as mybir
import concourse.tile as tile
from concourse.bass_utils import run_bass_kernel_spmd

F32 = mybir.dt.float32
FMAX = mybir.AluOpType.max

B, L, NK, KT, NC = 64, 2048, 2000, 9, 10
NCORES = 8
NKC = NK // NCORES  # 250 filters per core
LO = L - KT + 1  # 2040 valid conv positions
LPAD = 2112  # x padded row length
BASES = (0, 512, 1024, 1528)  # lo-block bases (last overlaps by 8)
CH = 8  # batches staged per x-tile chunk
NCHUNK = B // CH
NBLK = 2  # nk blocks per core: 128 + 122(pad->128)

_CACHE: dict = {}


def _register_max2r():
    """Custom DVE op: out = max(in0, in1), accum_out = reduce_max(out).

    Drains two 1024-wide halves of a PSUM group per lane-cycle — the native
    TENSOR_TENSOR_REDUCE / SCAN opcodes crash this runtime, but the custom
    DVE table path runs fine. in0 may be PSUM (only one PSUM input is legal
    per DVE instruction); in1 streams from SBUF."""
    import concourse.dve_ops as dve_ops
    from concourse.dve_ops import DveOp, has_src1
    from concourse.dve_spec import AluOp, Spec, Src0, Src1, lower, maxx
    from concourse.dve_uop import DveOpSpec

    for o in dve_ops.OPS:
        if o.name == "ANT_MAX2_REDUCE":
            return o

    def _ref(in0, in1, c0, c1, c2):
        m = np.maximum(in0, in1)
        return m, m.reshape(m.shape[0], -1).max(axis=-1, keepdims=True)

    spec = Spec(body=maxx(Src0, Src1), accum=AluOp.MAX, reference=_ref)
    op = DveOp("ANT_MAX2_REDUCE", spec, subdim=False, uops_sha={})
    dve_ops.OPS.append(op)
    dve_ops.CUSTOM_DVE_SPECS[op.name] = op.spec
    dve_ops._SUB_OPCODE_FOR_NAME[op.name] = (
        dve_ops._CUSTOM_DVE_ROW_BASE + len(dve_ops.OPS) - 1
    )
    for ver in ("v3", "v4"):
        s = DveOpSpec(
            name=op.name,
            opcode=dve_ops.get_dve_sub_opcode(op.name),
            uops=lower(spec, ver=ver),
            rd1_en=has_src1(spec),
        )
        op.uops_sha[ver] = s.sha(ver)
    return op


def _build_module():
    max2r = _register_max2r()
    nc = bacc.Bacc("TRN2", target_bir_lowering=False, debug=False)

    xp_t = nc.dram_tensor("xp", [B, LPAD], F32, kind="ExternalInput")
    wrep_t = nc.dram_tensor("wrep", [128, 256], F32, kind="ExternalInput")
    wmt_t = nc.dram_tensor("wmt", [256, NC], F32, kind="ExternalInput")
    maug_t = nc.dram_tensor("maug", [NC, NC], F32, kind="ExternalInput")
    outT_t = nc.dram_tensor("outT", [NC, B], F32, kind="ExternalOutput")

    xp = xp_t.ap()
    with tile.TileContext(nc) as tc, ExitStack() as ctx:
        wpool = ctx.enter_context(tc.tile_pool(name="wpool", bufs=1))
        xpool = ctx.enter_context(tc.tile_pool(name="xpool", bufs=3))
        pspool = ctx.enter_context(tc.tile_pool(name="pspool", bufs=2, space="PSUM"))
        fpool = ctx.enter_context(tc.tile_pool(name="fpool", bufs=3))
        dpool = ctx.enter_context(tc.tile_pool(name="dpool", bufs=1, space="DRAM"))

        # --- load constants/weights ---
        wt = wpool.tile([128, 256], F32)  # conv weights, 4x replicated row groups
        nc.sync.dma_start(wt[:, :], wrep_t.ap())
        wm0 = wpool.tile([128, NC], F32)
        nc.sync.dma_start(wm0[:, :], wmt_t.ap()[0:128, :])
        wm1 = wpool.tile([128, NC], F32)
        nc.sync.dma_start(wm1[:, :], wmt_t.ap()[128:256, :])
        mt = wpool.tile([NC, NC], F32)
        nc.sync.dma_start(mt[:, :], maug_t.ap())

        # --- S path state (mean pooling sums); ops emitted inside the conv
        # loop so the ACT engine interleaves them with PSUM-drain copies ---
        xr = wpool.tile([B, LPAD], F32)  # x in [batch-partition, col] layout
        nc.sync.dma_start(xr[:, :], xp)
        sgarb = wpool.tile([B, LO], F32)  # ACT copy target, values unused
        ssb = wpool.tile([B, NC], F32)  # S[b,k] for k<9; col 9 = 1.0 (bias row)
        nc.gpsimd.memset(ssb[:, KT : KT + 1], 1.0)

        def emit_s_op(k):
            nc.scalar.activation(
                sgarb[:, :],
                xr[:, k : k + LO],
                mybir.ActivationFunctionType.Copy,
                accum_out=ssb[:, k : k + 1],
            )

        # --- max features ---
        mf0 = wpool.tile([128, B], F32)
        mf1 = wpool.tile([128, B], F32)
        mfs = (mf0, mf1)

        unit_idx = 0
        for chunk in range(NCHUNK):
            # stage shifted x windows: partition 32g+k holds
            # x[b, BASES[g] + k + col] for col in [0,512)
            xt = xpool.tile([128, CH, 512], F32, tag="xt")
            for g in range(4):
                src = bass.AP(
                    xp.tensor,
                    chunk * CH * LPAD + BASES[g],
                    [[1, KT], [LPAD, CH], [1, 512]],
                )
                nc.sync.dma_start(xt[32 * g : 32 * g + KT, :, :], src)
            for blk in range(NBLK):
                for bl in range(CH):
                    b = chunk * CH + bl
                    ps = pspool.tile([128, 2048], F32, tag="ps")
                    for g in range(4):
                        nc.tensor.matmul(
                            ps[:, 512 * g : 512 * (g + 1)],
                            lhsT=wt[32 * g : 32 * g + KT, 128 * blk : 128 * (blk + 1)],
                            rhs=xt[32 * g : 32 * g + KT, bl, :],
                            start=True,
                            stop=True,
                            tile_position=(32 * g, 0),
                        )
                    # HW allows only ONE PSUM input per DVE/ACT instruction:
                    # ACT copies the low half to SBUF, then one custom DVE op
                    # max-folds it with the high PSUM half and reduces.
                    fold = fpool.tile([128, 1024], F32, tag="fold")
                    nc.scalar.copy(fold[:, :], ps[:, 0:1024])
                    tout = fpool.tile([128, 1024], F32, tag="tout")
                    nc.vector._custom_dve(
                        max2r,
                        out=tout[:, :],
                        in0=ps[:, 1024:2048],
                        in1=fold[:, :],
                        accum_out=mfs[blk][:, b : b + 1],
                    )
                    # spread the 9 mean-path ACT reductions through the loop
                    if unit_idx % 14 == 6 and unit_idx // 14 < KT:
                        emit_s_op(unit_idx // 14)
                    unit_idx += 1

        # transpose S [64,10] -> S.T [10,64] via a DRAM round-trip
        sdram = dpool.tile([B, NC], F32)
        nc.sync.dma_start(sdram[:, :], ssb[:, :])
        st = wpool.tile([NC, B], F32)
        nc.sync.dma_start(st[:, :], sdram.rearrange("b k -> k b"))

        # --- logits.T [10, 64] ---
        lg = pspool.tile([128, 2048], F32, tag="ps")
        nc.tensor.matmul(
            lg[0:NC, 0:B], lhsT=wm0[:, :], rhs=mf0[:, :],
            start=True, stop=False, tile_position=(0, 0),
        )
        nc.tensor.matmul(
            lg[0:NC, 0:B], lhsT=wm1[:, :], rhs=mf1[:, :],
            start=False, stop=False, tile_position=(0, 0),
        )
        nc.tensor.matmul(
            lg[0:NC, 0:B], lhsT=mt[:, :], rhs=st[:, :],
            start=False, stop=True, tile_position=(0, 0),
        )
        outsb = wpool.tile([NC, B], F32)
        nc.vector.tensor_copy(outsb[:, :], lg[0:NC, 0:B])
        nc.sync.dma_start(outT_t.ap(), outsb[:, :])

    nc.compile()
    return nc


def _prep_core_inputs(x, kern, W, b):
    """Host-side sharding + weight packing. Returns in_maps for 8 cores."""
    xp = np.zeros((B, LPAD), np.float32)
    xp[:, :L] = x
    in_maps = []
    for c in range(NCORES):
        ks = kern[c * NKC : (c + 1) * NKC]  # [250, 9]
        kpad = np.zeros((256, KT), np.float32)
        kpad[:NKC] = ks
        wrep = np.zeros((128, 256), np.float32)
        for g in range(4):
            wrep[32 * g : 32 * g + KT, 0:128] = kpad[0:128].T
            wrep[32 * g : 32 * g + KT, 128:256] = kpad[128:256].T
        wmax = W[:, 0::2][:, c * NKC : (c + 1) * NKC]  # [10, 250]
        wmt = np.zeros((256, NC), np.float32)
        wmt[:NKC] = wmax.T
        wmean = W[:, 1::2][:, c * NKC : (c + 1) * NKC]  # [10, 250]
        m = (wmean.astype(np.float64) @ ks.astype(np.float64)) / LO  # [10, 9]
        maug = np.zeros((NC, NC), np.float32)
        maug[0:KT, :] = m.T.astype(np.float32)
        maug[KT, :] = b / NCORES
        in_maps.append({"xp": xp, "wrep": wrep, "wmt": wmt, "maug": maug})
    return in_maps


def kernel(x, kernels, W, b, **kw):
    x = np.ascontiguousarray(np.asarray(x, np.float32).reshape(B, L))
    kern = np.ascontiguousarray(np.asarray(kernels, np.float32).reshape(NK, KT))
    W = np.asarray(W, np.float32)
    b = np.asarray(b, np.float32)

    if "nc" not in _CACHE:
        _CACHE["nc"] = _build_module()
    nc = _CACHE["nc"]

    in_maps = _prep_core_inputs(x, kern, W, b)
    res = run_bass_kernel_spmd(
        nc, in_maps, core_ids=list(range(NCORES)), **_CACHE.get("run_kwargs", {})
    )
    _CACHE["last_result"] = res
    out = np.zeros((B, NC), np.float64)
    for r in res.results:
        out += r["outT"].T.astype(np.float64)
    return out.astype(np.float32)


if __name__ == "__main__":
    rng = np.random.default_rng(0)
    out = kernel(
        x=rng.standard_normal((B, 1, L), dtype=np.float32),
        kernels=rng.standard_normal((NK, 1, KT), dtype=np.float32),
        W=rng.standard_normal((NC, 2 * NK), dtype=np.float32) * 0.02,
        b=np.zeros(NC, np.float32),
    )
    print(out.shape, out.dtype, out[:2, :4])


# revision 13
# speedup vs baseline: 272.4259x; 272.4259x over previous
"""Trainium2 Bass kernel for nn_DifferentiableRocket.

Model: y = [max_pool ‖ mean_pool](conv1d(x, kernels)) @ W.T + b
  x [64,1,2048] f32, kernels [2000,1,9], W [10,4000], b [10] -> out [64,10]

Sharding: kernel-axis tensor parallel — each of 8 cores owns 250 conv
filters and the matching classifier columns; partial logits are summed on
the host (cheaper than an on-device all-reduce for a [10,64] tile).

Per-core device algorithm:
  * conv as 4x row-tiled PE matmuls (contraction = 9 taps, one 32-row
    group per lo-block of 512), weights stationary, out in PSUM
    [128 nk, 2048 lo] per (batch, nk-block) unit.
  * max-pool: ONE DVE tensor_tensor_reduce per unit:
        fold = max(psum[:, :1024], psum[:, 1024:2048])
        maxfeat[:, b] = reduce_max(fold)
    which drains PSUM at the 2-elem/lane/cycle ceiling.
  * mean-pool is algebraically collapsed: mean-part of the logits equals
    (S @ M.T) where S[b,k] = sum_lo x[b, lo+k] (9 windowed sums, computed
    on the idle ACT engine via activation accum_out) and
    M[c,k] = sum_nk W[c,2nk+1] * kernels[nk,k] / 2040 (host-precomputed
    weight packing; bias/8 folded in as a 10th row).
  * logits.T [10,64] = W_max.T-matmuls over maxfeat + M_aug @ S_aug.T,
    accumulated in one PSUM tile, DMA'd out; host sums the 8 cores.
"""

import sys

sys.path.insert(0, "/opt/trn_rl_repo")

from contextlib import ExitStack

import numpy as np

import concourse.bacc as bacc
import concourse.bass as bass
import concourse.mybir as mybir
import concourse.tile as tile
from concourse.bass_utils import run_bass_kernel_spmd

F32 = mybir.dt.float32
FMAX = mybir.AluOpType.max

B, L, NK, KT, NC = 64, 2048, 2000, 9, 10
NCORES = 8
NKC = NK // NCORES  # 250 filters per core
LO = L - KT + 1  # 2040 valid conv positions
LPAD = 2112  # x padded row length
BASES = (0, 512, 1024, 1528)  # lo-block bases (last overlaps by 8)
CH = 8  # batches staged per x-tile chunk
NCHUNK = B // CH
NBLK = 2  # nk blocks per core: 128 + 122(pad->128)

_CACHE: dict = {}


def _register_max2r():
    """Custom DVE op: out = max(in0, in1), accum_out = reduce_max(out).

    Drains two 1024-wide halves of a PSUM group per lane-cycle — the native
    TENSOR_TENSOR_REDUCE / SCAN opcodes crash this runtime, but the custom
    DVE table path runs fine. in0 may be PSUM (only one PSUM input is legal
    per DVE instruction); in1 streams from SBUF."""
    import concourse.dve_ops as dve_ops
    from concourse.dve_ops import DveOp, has_src1
    from concourse.dve_spec import AluOp, Spec, Src0, Src1, lower, maxx
    from concourse.dve_uop import DveOpSpec

    for o in dve_ops.OPS:
        if o.name == "ANT_MAX2_REDUCE":
            return o

    def _ref(in0, in1, c0, c1, c2):
        m = np.maximum(in0, in1)
        return m, m.reshape(m.shape[0], -1).max(axis=-1, keepdims=True)

    spec = Spec(body=maxx(Src0, Src1), accum=AluOp.MAX, reference=_ref)
    op = DveOp("ANT_MAX2_REDUCE", spec, subdim=False, uops_sha={})
    dve_ops.OPS.append(op)
    dve_ops.CUSTOM_DVE_SPECS[op.name] = op.spec
    dve_ops._SUB_OPCODE_FOR_NAME[op.name] = (
        dve_ops._CUSTOM_DVE_ROW_BASE + len(dve_ops.OPS) - 1
    )
    for ver in ("v3", "v4"):
        s = DveOpSpec(
            name=op.name,
            opcode=dve_ops.get_dve_sub_opcode(op.name),
            uops=lower(spec, ver=ver),
            rd1_en=has_src1(spec),
        )
        op.uops_sha[ver] = s.sha(ver)
    return op


def _build_module(device_reps: int = 1, skip_drain: bool = False,
                  skip_pe: bool = False):
    max2r = _register_max2r()
    nc = bacc.Bacc("TRN2", target_bir_lowering=False, debug=False)

    xp_t = nc.dram_tensor("xp", [B, LPAD], F32, kind="ExternalInput")
    wrep_t = nc.dram_tensor("wrep", [128, 256], F32, kind="ExternalInput")
    wmt_t = nc.dram_tensor("wmt", [256, NC], F32, kind="ExternalInput")
    maug_t = nc.dram_tensor("maug", [NC, NC], F32, kind="ExternalInput")
    outT_t = nc.dram_tensor("outT", [NC, B], F32, kind="ExternalOutput")

    xp = xp_t.ap()
    with tile.TileContext(nc) as tc, ExitStack() as ctx:
        wpool = ctx.enter_context(tc.tile_pool(name="wpool", bufs=1))
        xpool = ctx.enter_context(tc.tile_pool(name="xpool", bufs=3))
        pspool = ctx.enter_context(tc.tile_pool(name="pspool", bufs=2, space="PSUM"))
        fpool = ctx.enter_context(tc.tile_pool(name="fpool", bufs=3))
        dpool = ctx.enter_context(tc.tile_pool(name="dpool", bufs=1, space="DRAM"))

        # --- load constants/weights ---
        wt = wpool.tile([128, 256], F32)  # conv weights, 4x replicated row groups
        nc.sync.dma_start(wt[:, :], wrep_t.ap())
        wm0 = wpool.tile([128, NC], F32)
        nc.sync.dma_start(wm0[:, :], wmt_t.ap()[0:128, :])
        wm1 = wpool.tile([128, NC], F32)
        nc.sync.dma_start(wm1[:, :], wmt_t.ap()[128:256, :])
        mt = wpool.tile([NC, NC], F32)
        nc.sync.dma_start(mt[:, :], maug_t.ap())

        # --- S path state (mean pooling sums); ops emitted inside the conv
        # loop so the ACT engine interleaves them with PSUM-drain copies ---
        xr = wpool.tile([B, LPAD], F32)  # x in [batch-partition, col] layout
        nc.sync.dma_start(xr[:, :], xp)
        sgarb = wpool.tile([B, LO], F32)  # ACT copy target, values unused
        ssb = wpool.tile([B, NC], F32)  # S[b,k] for k<9; col 9 = 1.0 (bias row)
        nc.gpsimd.memset(ssb[:, KT : KT + 1], 1.0)

        def emit_s_op(k):
            nc.scalar.activation(
                sgarb[:, :],
                xr[:, k : k + LO],
                mybir.ActivationFunctionType.Copy,
                accum_out=ssb[:, k : k + 1],
            )

        # --- max features ---
        mf0 = wpool.tile([128, B], F32)
        mf1 = wpool.tile([128, B], F32)
        mfs = (mf0, mf1)

        unit_idx = 0
        for _rep in range(device_reps):
            for chunk in range(NCHUNK):
                # stage shifted x windows: partition 32g+k holds
                # x[b, BASES[g] + k + col] for col in [0,512)
                xt = xpool.tile([128, CH, 512], F32, tag="xt")
                for g in range(4):
                    src = bass.AP(
                        xp.tensor,
                        chunk * CH * LPAD + BASES[g],
                        [[1, KT], [LPAD, CH], [1, 512]],
                    )
                    nc.sync.dma_start(xt[32 * g : 32 * g + KT, :, :], src)
                for blk in range(NBLK):
                    for bl in range(CH):
                        b = chunk * CH + bl
                        ps = pspool.tile([128, 2048], F32, tag="ps")
                        if not skip_pe:
                            for g in range(4):
                                nc.tensor.matmul(
                                    ps[:, 512 * g : 512 * (g + 1)],
                                    lhsT=wt[
                                        32 * g : 32 * g + KT,
                                        128 * blk : 128 * (blk + 1),
                                    ],
                                    rhs=xt[32 * g : 32 * g + KT, bl, :],
                                    start=True,
                                    stop=True,
                                    tile_position=(32 * g, 0),
                                )
                        if skip_drain:
                            continue
                        # HW allows only ONE PSUM input per DVE/ACT inst:
                        # ACT copies the low half to SBUF, then one custom
                        # DVE op max-folds it with the high PSUM half and
                        # reduces.
                        fold = fpool.tile([128, 1024], F32, tag="fold")
                        nc.scalar.copy(fold[:, :], ps[:, 0:1024])
                        tout = fpool.tile([128, 1024], F32, tag="tout")
                        nc.vector._custom_dve(
                            max2r,
                            out=tout[:, :],
                            in0=ps[:, 1024:2048],
                            in1=fold[:, :],
                            accum_out=mfs[blk][:, b : b + 1],
                        )
                        # spread the 9 mean-path ACT ops through the loop
                        if unit_idx % 14 == 6 and unit_idx // 14 < KT:
                            emit_s_op(unit_idx // 14)
                        unit_idx += 1

        # transpose S [64,10] -> S.T [10,64] via a DRAM round-trip
        sdram = dpool.tile([B, NC], F32)
        nc.sync.dma_start(sdram[:, :], ssb[:, :])
        st = wpool.tile([NC, B], F32)
        nc.sync.dma_start(st[:, :], sdram.rearrange("b k -> k b"))

        # --- logits.T [10, 64] ---
        lg = pspool.tile([128, 2048], F32, tag="ps")
        nc.tensor.matmul(
            lg[0:NC, 0:B], lhsT=wm0[:, :], rhs=mf0[:, :],
            start=True, stop=False, tile_position=(0, 0),
        )
        nc.tensor.matmul(
            lg[0:NC, 0:B], lhsT=wm1[:, :], rhs=mf1[:, :],
            start=False, stop=False, tile_position=(0, 0),
        )
        nc.tensor.matmul(
            lg[0:NC, 0:B], lhsT=mt[:, :], rhs=st[:, :],
            start=False, stop=True, tile_position=(0, 0),
        )
        outsb = wpool.tile([NC, B], F32)
        nc.vector.tensor_copy(outsb[:, :], lg[0:NC, 0:B])
        nc.sync.dma_start(outT_t.ap(), outsb[:, :])

    nc.compile()
    return nc


def _prep_core_inputs(x, kern, W, b):
    """Host-side sharding + weight packing. Returns in_maps for 8 cores."""
    xp = np.zeros((B, LPAD), np.float32)
    xp[:, :L] = x
    in_maps = []
    for c in range(NCORES):
        ks = kern[c * NKC : (c + 1) * NKC]  # [250, 9]
        kpad = np.zeros((256, KT), np.float32)
        kpad[:NKC] = ks
        wrep = np.zeros((128, 256), np.float32)
        for g in range(4):
            wrep[32 * g : 32 * g + KT, 0:128] = kpad[0:128].T
            wrep[32 * g : 32 * g + KT, 128:256] = kpad[128:256].T
        wmax = W[:, 0::2][:, c * NKC : (c + 1) * NKC]  # [10, 250]
        wmt = np.zeros((256, NC), np.float32)
        wmt[:NKC] = wmax.T
        wmean = W[:, 1::2][:, c * NKC : (c + 1) * NKC]  # [10, 250]
        m = (wmean.astype(np.float64) @ ks.astype(np.float64)) / LO  # [10, 9]
        maug = np.zeros((NC, NC), np.float32)
        maug[0:KT, :] = m.T.astype(np.float32)
        maug[KT, :] = b / NCORES
        in_maps.append({"xp": xp, "wrep": wrep, "wmt": wmt, "maug": maug})
    return in_maps


def kernel(x, kernels, W, b, **kw):
    x = np.ascontiguousarray(np.asarray(x, np.float32).reshape(B, L))
    kern = np.ascontiguousarray(np.asarray(kernels, np.float32).reshape(NK, KT))
    W = np.asarray(W, np.float32)
    b = np.asarray(b, np.float32)

    if "nc" not in _CACHE:
        _CACHE["nc"] = _build_module()
    nc = _CACHE["nc"]

    in_maps = _prep_core_inputs(x, kern, W, b)
    res = run_bass_kernel_spmd(
        nc, in_maps, core_ids=list(range(NCORES)), **_CACHE.get("run_kwargs", {})
    )
    _CACHE["last_result"] = res
    out = np.zeros((B, NC), np.float64)
    for r in res.results:
        out += r["outT"].T.astype(np.float64)
    return out.astype(np.float32)


if __name__ == "__main__":
    rng = np.random.default_rng(0)
    out = kernel(
        x=rng.standard_normal((B, 1, L), dtype=np.float32),
        kernels=rng.standard_normal((NK, 1, KT), dtype=np.float32),
        W=rng.standard_normal((NC, 2 * NK), dtype=np.float32) * 0.02,
        b=np.zeros(NC, np.float32),
    )
    print(out.shape, out.dtype, out[:2, :4])


# revision 15
# speedup vs baseline: 291.7430x; 1.0709x over previous
"""Trainium2 Bass kernel for nn_DifferentiableRocket.

Model: y = [max_pool ‖ mean_pool](conv1d(x, kernels)) @ W.T + b
  x [64,1,2048] f32, kernels [2000,1,9], W [10,4000], b [10] -> out [64,10]

Sharding: kernel-axis tensor parallel — each of 8 cores owns 250 conv
filters and the matching classifier columns; partial logits are summed on
the host (cheaper than an on-device all-reduce for a [10,64] tile).

Per-core device algorithm:
  * conv as 4x row-tiled PE matmuls (contraction = 9 taps, one 32-row
    group per lo-block of 512), weights stationary, out in PSUM
    [128 nk, 2048 lo] per (batch, nk-block) unit.
  * max-pool drain per unit: ACT copies psum[:, :1024] to SBUF (one PSUM
    input per instruction is a HW rule), then ONE custom DVE op
    (ANT_MAX2_REDUCE: out = max(in0, in1), accum_out = reduce_max(out))
    merges the high PSUM half with the copy and reduces — draining PSUM at
    the DVE's 2-elem/lane/cycle ceiling. (The native TENSOR_TENSOR_REDUCE
    opcode crashes this runtime; the custom DVE table path works.)
  * mean-pool is algebraically collapsed: mean-part of the logits equals
    (S @ M.T) where S[b,k] = sum_lo x[b, lo+k] (9 windowed sums, computed
    on the idle ACT engine via activation accum_out) and
    M[c,k] = sum_nk W[c,2nk+1] * kernels[nk,k] / 2040 (host-precomputed
    weight packing; bias/8 folded in as a 10th row).
  * logits.T [10,64] = W_max.T-matmuls over maxfeat + M_aug @ S_aug.T,
    accumulated in one PSUM tile, DMA'd out; host sums the 8 cores.
"""

import sys

sys.path.insert(0, "/opt/trn_rl_repo")

from contextlib import ExitStack

import numpy as np

import concourse.bacc as bacc
import concourse.bass as bass
import concourse.mybir as mybir
import concourse.tile as tile
from concourse.bass_utils import run_bass_kernel_spmd

F32 = mybir.dt.float32
FMAX = mybir.AluOpType.max

B, L, NK, KT, NC = 64, 2048, 2000, 9, 10
NCORES = 8
NKC = NK // NCORES  # 250 filters per core
LO = L - KT + 1  # 2040 valid conv positions
LPAD = 2112  # x padded row length
BASES = (0, 512, 1024, 1528)  # lo-block bases (last overlaps by 8)
CH = 8  # batches staged per x-tile chunk
NCHUNK = B // CH
NBLK = 2  # nk blocks per core: 128 + 122(pad->128)

_CACHE: dict = {}


def _register_max2r():
    """Custom DVE op: out = max(in0, in1), accum_out = reduce_max(out).

    Drains two 1024-wide halves of a PSUM group per lane-cycle — the native
    TENSOR_TENSOR_REDUCE / SCAN opcodes crash this runtime, but the custom
    DVE table path runs fine. in0 may be PSUM (only one PSUM input is legal
    per DVE instruction); in1 streams from SBUF."""
    import concourse.dve_ops as dve_ops
    from concourse.dve_ops import DveOp, has_src1
    from concourse.dve_spec import AluOp, Spec, Src0, Src1, lower, maxx
    from concourse.dve_uop import DveOpSpec

    for o in dve_ops.OPS:
        if o.name == "ANT_MAX2_REDUCE":
            return o

    def _ref(in0, in1, c0, c1, c2):
        m = np.maximum(in0, in1)
        return m, m.reshape(m.shape[0], -1).max(axis=-1, keepdims=True)

    spec = Spec(body=maxx(Src0, Src1), accum=AluOp.MAX, reference=_ref)
    op = DveOp("ANT_MAX2_REDUCE", spec, subdim=False, uops_sha={})
    dve_ops.OPS.append(op)
    dve_ops.CUSTOM_DVE_SPECS[op.name] = op.spec
    dve_ops._SUB_OPCODE_FOR_NAME[op.name] = (
        dve_ops._CUSTOM_DVE_ROW_BASE + len(dve_ops.OPS) - 1
    )
    for ver in ("v3", "v4"):
        s = DveOpSpec(
            name=op.name,
            opcode=dve_ops.get_dve_sub_opcode(op.name),
            uops=lower(spec, ver=ver),
            rd1_en=has_src1(spec),
        )
        op.uops_sha[ver] = s.sha(ver)
    return op


def _build_module(device_reps: int = 1, skip_drain: bool = False,
                  skip_pe: bool = False):
    max2r = _register_max2r()
    nc = bacc.Bacc("TRN2", target_bir_lowering=False, debug=False)

    xp_t = nc.dram_tensor("xp", [B, LPAD], F32, kind="ExternalInput")
    wrep_t = nc.dram_tensor("wrep", [128, 256], F32, kind="ExternalInput")
    wmt_t = nc.dram_tensor("wmt", [256, NC], F32, kind="ExternalInput")
    maug_t = nc.dram_tensor("maug", [NC, NC], F32, kind="ExternalInput")
    outT_t = nc.dram_tensor("outT", [NC, B], F32, kind="ExternalOutput")

    xp = xp_t.ap()
    with tile.TileContext(nc) as tc, ExitStack() as ctx:
        wpool = ctx.enter_context(tc.tile_pool(name="wpool", bufs=1))
        xpool = ctx.enter_context(tc.tile_pool(name="xpool", bufs=4))
        pspool = ctx.enter_context(tc.tile_pool(name="pspool", bufs=2, space="PSUM"))
        fpool = ctx.enter_context(tc.tile_pool(name="fpool", bufs=6))
        dpool = ctx.enter_context(tc.tile_pool(name="dpool", bufs=1, space="DRAM"))

        # --- load constants/weights ---
        wt = wpool.tile([128, 256], F32)  # conv weights, 4x replicated row groups
        nc.sync.dma_start(wt[:, :], wrep_t.ap())
        wm0 = wpool.tile([128, NC], F32)
        nc.sync.dma_start(wm0[:, :], wmt_t.ap()[0:128, :])
        wm1 = wpool.tile([128, NC], F32)
        nc.sync.dma_start(wm1[:, :], wmt_t.ap()[128:256, :])
        mt = wpool.tile([NC, NC], F32)
        nc.sync.dma_start(mt[:, :], maug_t.ap())

        # --- S path state (mean pooling sums); ops emitted inside the conv
        # loop so the ACT engine interleaves them with PSUM-drain copies ---
        xr = wpool.tile([B, LPAD], F32)  # x in [batch-partition, col] layout
        nc.sync.dma_start(xr[:, :], xp)
        sgarb = wpool.tile([B, LO], F32)  # ACT copy target, values unused
        ssb = wpool.tile([B, NC], F32)  # S[b,k] for k<9; col 9 = 1.0 (bias row)
        nc.gpsimd.memset(ssb[:, KT : KT + 1], 1.0)

        def emit_s_op(k):
            nc.scalar.activation(
                sgarb[:, :],
                xr[:, k : k + LO],
                mybir.ActivationFunctionType.Copy,
                accum_out=ssb[:, k : k + 1],
            )

        # --- max features ---
        mf0 = wpool.tile([128, B], F32)
        mf1 = wpool.tile([128, B], F32)
        mfs = (mf0, mf1)

        unit_idx = 0
        for _rep in range(device_reps):
            for chunk in range(NCHUNK):
                # stage shifted x windows: partition 32g+k holds
                # x[b, BASES[g] + k + col] for col in [0,512)
                xt = xpool.tile([128, CH, 512], F32, tag="xt")
                for g in range(4):
                    src = bass.AP(
                        xp.tensor,
                        chunk * CH * LPAD + BASES[g],
                        [[1, KT], [LPAD, CH], [1, 512]],
                    )
                    nc.sync.dma_start(xt[32 * g : 32 * g + KT, :, :], src)
                for blk in range(NBLK):
                    for bl in range(CH):
                        b = chunk * CH + bl
                        ps = pspool.tile([128, 2048], F32, tag="ps")
                        if not skip_pe:
                            for g in range(4):
                                nc.tensor.matmul(
                                    ps[:, 512 * g : 512 * (g + 1)],
                                    lhsT=wt[
                                        32 * g : 32 * g + KT,
                                        128 * blk : 128 * (blk + 1),
                                    ],
                                    rhs=xt[32 * g : 32 * g + KT, bl, :],
                                    start=True,
                                    stop=True,
                                    tile_position=(32 * g, 0),
                                )
                        if skip_drain:
                            continue
                        # HW allows only ONE PSUM input per DVE/ACT inst:
                        # ACT copies the low half to SBUF, then one custom
                        # DVE op max-folds it with the high PSUM half and
                        # reduces.
                        fold = fpool.tile([128, 1024], F32, tag="fold")
                        nc.scalar.copy(fold[:, :], ps[:, 0:1024])
                        tout = fpool.tile([128, 1024], F32, tag="tout")
                        nc.vector._custom_dve(
                            max2r,
                            out=tout[:, :],
                            in0=ps[:, 1024:2048],
                            in1=fold[:, :],
                            accum_out=mfs[blk][:, b : b + 1],
                        )
                        # spread the 9 mean-path ACT ops through the loop
                        if unit_idx % 14 == 6 and unit_idx // 14 < KT:
                            emit_s_op(unit_idx // 14)
                        unit_idx += 1

        # transpose S [64,10] -> S.T [10,64] via a DRAM round-trip
        sdram = dpool.tile([B, NC], F32)
        nc.sync.dma_start(sdram[:, :], ssb[:, :])
        st = wpool.tile([NC, B], F32)
        nc.sync.dma_start(st[:, :], sdram.rearrange("b k -> k b"))

        # --- logits.T [10, 64] ---
        lg = pspool.tile([128, 2048], F32, tag="ps")
        nc.tensor.matmul(
            lg[0:NC, 0:B], lhsT=wm0[:, :], rhs=mf0[:, :],
            start=True, stop=False, tile_position=(0, 0),
        )
        nc.tensor.matmul(
            lg[0:NC, 0:B], lhsT=wm1[:, :], rhs=mf1[:, :],
            start=False, stop=False, tile_position=(0, 0),
        )
        nc.tensor.matmul(
            lg[0:NC, 0:B], lhsT=mt[:, :], rhs=st[:, :],
            start=False, stop=True, tile_position=(0, 0),
        )
        outsb = wpool.tile([NC, B], F32)
        nc.vector.tensor_copy(outsb[:, :], lg[0:NC, 0:B])
        nc.sync.dma_start(outT_t.ap(), outsb[:, :])

    nc.compile()
    return nc


def _prep_core_inputs(x, kern, W, b):
    """Host-side sharding + weight packing. Returns in_maps for 8 cores."""
    xp = np.zeros((B, LPAD), np.float32)
    xp[:, :L] = x
    in_maps = []
    for c in range(NCORES):
        ks = kern[c * NKC : (c + 1) * NKC]  # [250, 9]
        kpad = np.zeros((256, KT), np.float32)
        kpad[:NKC] = ks
        wrep = np.zeros((128, 256), np.float32)
        for g in range(4):
            wrep[32 * g : 32 * g + KT, 0:128] = kpad[0:128].T
            wrep[32 * g : 32 * g + KT, 128:256] = kpad[128:256].T
        wmax = W[:, 0::2][:, c * NKC : (c + 1) * NKC]  # [10, 250]
        wmt = np.zeros((256, NC), np.float32)
        wmt[:NKC] = wmax.T
        wmean = W[:, 1::2][:, c * NKC : (c + 1) * NKC]  # [10, 250]
        m = (wmean.astype(np.float64) @ ks.astype(np.float64)) / LO  # [10, 9]
        maug = np.zeros((NC, NC), np.float32)
        maug[0:KT, :] = m.T.astype(np.float32)
        maug[KT, :] = b / NCORES
        in_maps.append({"xp": xp, "wrep": wrep, "wmt": wmt, "maug": maug})
    return in_maps


def kernel(x, kernels, W, b, **kw):
    x = np.ascontiguousarray(np.asarray(x, np.float32).reshape(B, L))
    kern = np.ascontiguousarray(np.asarray(kernels, np.float32).reshape(NK, KT))
    W = np.asarray(W, np.float32)
    b = np.asarray(b, np.float32)

    if "nc" not in _CACHE:
        _CACHE["nc"] = _build_module()
    nc = _CACHE["nc"]

    in_maps = _prep_core_inputs(x, kern, W, b)
    res = run_bass_kernel_spmd(
        nc, in_maps, core_ids=list(range(NCORES)), **_CACHE.get("run_kwargs", {})
    )
    _CACHE["last_result"] = res
    out = np.zeros((B, NC), np.float64)
    for r in res.results:
        out += r["outT"].T.astype(np.float64)
    return out.astype(np.float32)


if __name__ == "__main__":
    rng = np.random.default_rng(0)
    out = kernel(
        x=rng.standard_normal((B, 1, L), dtype=np.float32),
        kernels=rng.standard_normal((NK, 1, KT), dtype=np.float32),
        W=rng.standard_normal((NC, 2 * NK), dtype=np.float32) * 0.02,
        b=np.zeros(NC, np.float32),
    )
    print(out.shape, out.dtype, out[:2, :4])
